# revision 1
# baseline (speedup 1.0000x reference)
"""MoE layer (top-2 of 8 experts, d_model=1024, d_hidden=512) on 8 trn2 cores.

Token-parallel: each core processes 1024 of the 8192 tokens against all 8
experts. Gating (logits, top-2, softmax) is computed on-device in fp32;
the two expert MLP matmuls run in fp32r (full PE speed). The gate weight is
folded into the combine step as a per-partition scalar multiply-accumulate,
so non-selected experts contribute 0 exactly as in the reference math.

Layout notes:
  - x arrives host-transposed per-shard as xT [D, TC] so both MLP matmuls can
    contract over the partition dimension with weights in native layout.
  - mm1 produces hT [C, tokens] (expert weights stationary), mm2 flips back to
    token-major y [tokens, D] (hT chunks stationary) so the gate is a
    per-partition [128,1] scalar and the output DMAs out in native layout.
"""

import os
import sys

import numpy as np

for _p in ("/opt/trn_rl_repo", "/root/.axon_site/_ro/trn_rl_repo"):
    if _p not in sys.path and os.path.isdir(_p):
        sys.path.append(_p)

P = 128
D_MODEL = 1024
C_HID = 512
N_EXP = 8
TOP_K = 2
N_CORES = 8
T_FULL = 4 * 2048
TC = T_FULL // N_CORES  # tokens per core

KC = D_MODEL // P  # 8 contraction chunks over D
CC = C_HID // P    # 4 contraction chunks over C
TT = TC // P       # 8 token chunks of 128
NT = 512           # moving-dim chunk (tokens) for mm1
DH = 512           # moving-dim chunk (d_model) for mm2

_CACHE = {}

# set by test harness to capture profiling info
TRACE = False
LAST_RESULT = None


def _install_ntff_hook_shim():
    """Register the axon NTFF profile hook if the image's antenv lacks it.

    bass_utils resolves the hook via `antenv.axon_hooks`; when that module is
    absent, tracing silently degrades. The hook implementation itself ships
    with the axon boot package, so wire it up through sys.modules.
    """
    try:
        from antenv.axon_hooks import get_axon_ntff_profile_hook  # noqa: F401
        return  # real module present
    except ImportError:
        pass
    try:
        import types

        if "/root/.axon_site" not in sys.path and os.path.isdir("/root/.axon_site"):
            sys.path.append("/root/.axon_site")
        from trn_agent_boot.trn_boot import _ntff_profile_via_ctypes

        so_path = "/opt/axon/libaxon_pjrt.so"
        if not os.path.exists(so_path):
            return
        hook = _ntff_profile_via_ctypes(so_path)
        mod = types.ModuleType("antenv.axon_hooks")
        mod.get_axon_ntff_profile_hook = lambda: hook
        mod.set_axon_ntff_profile_hook = lambda h: None
        import antenv

        antenv.axon_hooks = mod
        sys.modules["antenv.axon_hooks"] = mod
    except Exception:
        pass


def _split_excess_waits(nc, mybir, maxw=1):
    """This walrus build accepts at most one semaphore wait per instruction.

    Tile emits instructions (notably the kernel-tail drain) with several
    waits; split the extras into preceding single-wait NoOps on the same
    engine — program order makes the chain equivalent.
    """
    for f in nc.m.functions:
        for bb in f.blocks:
            out = []
            changed = False
            for ins in bb.instructions:
                si = ins.sync_info
                waits = list(si.on_wait) if (si is not None and si.on_wait) else []
                if len(waits) > maxw:
                    extra, keep = waits[:-maxw], waits[-maxw:]
                    for ci in range(0, len(extra), maxw):
                        out.append(mybir.InstNoOp(
                            name=f"{ins.name}_ws{ci}",
                            sync_info=mybir.SyncInfo(
                                on_wait=list(extra[ci:ci + maxw]), on_update=[]
                            ),
                            engine=ins.engine,
                            bass_nofuse=True,
                        ))
                    si.on_wait = keep
                    changed = True
                out.append(ins)
            if changed:
                bb.instructions = out


def _build_nc():
    import concourse.bass as bass
    import concourse.mybir as mybir
    import concourse.tile as tile
    from contextlib import ExitStack

    dt = mybir.dt
    f32 = dt.float32
    f32r = dt.float32r
    f16 = dt.float16
    AX = mybir.AxisListType
    OP = mybir.AluOpType
    ACT = mybir.ActivationFunctionType

    nc = bass.Bass("TRN2", debug=False)

    xT = nc.dram_tensor("xT", [D_MODEL, TC], f16, kind="ExternalInput")
    dxT = nc.dram_tensor("dxT", [D_MODEL, TC], f16, kind="ExternalInput")
    wgp = nc.dram_tensor("wgp", [D_MODEL, 2 * N_EXP], f16, kind="ExternalInput")
    w1 = nc.dram_tensor("w1", [N_EXP, D_MODEL, C_HID], f16, kind="ExternalInput")
    w2 = nc.dram_tensor("w2", [N_EXP, C_HID, D_MODEL], f16, kind="ExternalInput")
    id8 = nc.dram_tensor("id8", [2 * N_EXP, 2 * N_EXP], f32, kind="ExternalInput")
    out = nc.dram_tensor("out", [TC, D_MODEL], f32, kind="ExternalOutput")

    with tile.TileContext(nc) as tc:
        with ExitStack() as ctx:
            cpool = ctx.enter_context(tc.tile_pool(name="cpool", bufs=1))
            wpool = ctx.enter_context(tc.tile_pool(name="wpool", bufs=2))
            hpool = ctx.enter_context(tc.tile_pool(name="hpool", bufs=2))
            gpool = ctx.enter_context(tc.tile_pool(name="gpool", bufs=2))
            psum_mm = ctx.enter_context(tc.tile_pool(name="psum_mm", bufs=4, space="PSUM"))
            psum_sm = ctx.enter_context(tc.tile_pool(name="psum_sm", bufs=3, space="PSUM"))

            xt_sb = cpool.tile([P, KC, TC], f16, name="xt_sb")
            dxt_sb = cpool.tile([P, KC, TC], f16, name="dxt_sb")
            wg_sb = cpool.tile([P, KC, 2 * N_EXP], f16, name="wg_sb")
            out_sb = cpool.tile([P, TT, D_MODEL], f32, name="out_sb")
            gate_sb = cpool.tile([P, TT, N_EXP], f32, name="gate_sb")
            id16_sb = cpool.tile([2 * N_EXP, 2 * N_EXP], f32, name="id16_sb")
            lgT_sb = cpool.tile([P, 2, TC], f32, name="lgT_sb")

            # DMA order tuned for earliest PE start: expert-0 weights and the
            # fp16 activations feed mm1(e0); the fp32 gating inputs follow in
            # small chunks so logits stream in behind it.
            w1_sb0 = wpool.tile([P, KC, C_HID], f16, name="w1_sb", tag="w1")
            w1r0 = w1[0].rearrange("(kc p) c -> p kc c", p=P)
            nc.sync.dma_start(w1_sb0[:, :, 0:P], w1r0[:, :, 0:P])
            nc.sync.dma_start(
                xt_sb[:, :, 0:NT],
                xT[:, 0:NT].rearrange("(kc p) t -> p kc t", p=P))
            for q in range(1, CC):
                nc.sync.dma_start(
                    w1_sb0[:, :, q * P:(q + 1) * P], w1r0[:, :, q * P:(q + 1) * P])
            nc.sync.dma_start(
                xt_sb[:, :, NT:TC],
                xT[:, NT:TC].rearrange("(kc p) t -> p kc t", p=P))
            w2_sb0 = wpool.tile([P, CC, D_MODEL], f16, name="w2_sb", tag="w2")
            nc.sync.dma_start(
                w2_sb0[:], w2[0].rearrange("(cc p) d -> p cc d", p=P))
            nc.sync.dma_start(wg_sb[:], wgp[:].rearrange("(kc p) e -> p kc e", p=P))
            nc.sync.dma_start(id16_sb[:], id8[:])
            for th2 in range(2):
                sl = slice(th2 * NT, (th2 + 1) * NT)
                nc.sync.dma_start(
                    dxt_sb[:, :, sl],
                    dxT[:, sl].rearrange("(kc p) t -> p kc t", p=P))

            def emit_mm1(w1_sb):
                ht_sb = hpool.tile([P, CC, TC], f16, name="ht_sb", tag="ht")
                for th in range(TC // NT):
                    for cm in range(CC):
                        ps_h = psum_mm.tile([P, NT], f32, name="ps_h", tag="ps")
                        for kc in range(KC):
                            nc.tensor.matmul(
                                ps_h[:],
                                lhsT=w1_sb[:, kc, cm * P:(cm + 1) * P],
                                rhs=xt_sb[:, kc, th * NT:(th + 1) * NT],
                                start=(kc == 0),
                                stop=(kc == KC - 1),
                            )
                        nc.scalar.activation(
                            ht_sb[:, cm, th * NT:(th + 1) * NT], ps_h[:], ACT.Relu
                        )
                return ht_sb

            def emit_mm2(e, w2_sb, ht_sb):
                for tt in range(TT):
                    for dh in range(D_MODEL // DH):
                        ps_y = psum_mm.tile([P, DH], f32, name="ps_y", tag="ps")
                        for cc in range(CC):
                            nc.tensor.matmul(
                                ps_y[:],
                                lhsT=ht_sb[:, cc, tt * P:(tt + 1) * P],
                                rhs=w2_sb[:, cc, dh * DH:(dh + 1) * DH],
                                start=(cc == 0),
                                stop=(cc == CC - 1),
                            )
                        o_sl = out_sb[:, tt, dh * DH:(dh + 1) * DH]
                        g_col = gate_sb[:, tt, e:e + 1]
                        if e == 0:
                            nc.vector.tensor_single_scalar(
                                o_sl, ps_y[:], g_col, op=OP.mult
                            )
                        else:
                            nc.vector.scalar_tensor_tensor(
                                o_sl, in0=ps_y[:], scalar=g_col, in1=o_sl,
                                op0=OP.mult, op1=OP.add,
                            )

            # expert-0 mm1 first in the PE stream (its inputs land first)
            ht_sb0 = emit_mm1(w1_sb0)

            # ---- routing: logitsT = [wg16|dwg].T @ x16 (+ dx correction into
            # rows 0:8), transposed back per chunk; top-2/softmax batched
            # across all 8 token chunks.
            logits_all = cpool.tile([P, TT, N_EXP], f32, name="logits_all")
            for th in range(2):
                ps_lt = psum_mm.tile([P, NT], f32, name="ps_lt", tag="ps")
                for kc in range(KC):
                    nc.tensor.matmul(
                        ps_lt[0:2 * N_EXP, :],
                        lhsT=wg_sb[:, kc, :],
                        rhs=xt_sb[:, kc, th * NT:(th + 1) * NT],
                        start=(kc == 0),
                        stop=False,
                    )
                for kc in range(KC):
                    nc.tensor.matmul(
                        ps_lt[0:N_EXP, :],
                        lhsT=wg_sb[:, kc, 0:N_EXP],
                        rhs=dxt_sb[:, kc, th * NT:(th + 1) * NT],
                        start=False,
                        stop=(kc == KC - 1),
                    )
                nc.vector.tensor_copy(
                    lgT_sb[0:2 * N_EXP, 0, th * NT:(th + 1) * NT],
                    ps_lt[0:2 * N_EXP, :])
            for tt in range(TT):
                ps_l = psum_sm.tile([P, 2 * N_EXP], f32, name="ps_l", tag="ps_l")
                nc.tensor.transpose(
                    ps_l[:], lgT_sb[0:2 * N_EXP, 0, tt * P:(tt + 1) * P], id16_sb[:])
                lgh = gpool.tile([P, N_EXP], f32, name="lgh", tag="lgh")
                nc.vector.tensor_copy(lgh[:], ps_l[:, N_EXP:2 * N_EXP])
                nc.vector.tensor_add(
                    logits_all[:, tt, :], ps_l[:, 0:N_EXP], lgh[:])

            def b3(ap2d):
                return ap2d.rearrange("p (t o) -> p t o", o=1).to_broadcast(
                    [P, TT, N_EXP])

            m1a = gpool.tile([P, TT], f32, name="m1a", tag="m1a", bufs=1)
            nc.vector.reduce_max(m1a[:], logits_all[:], axis=AX.X)
            eq1a = gpool.tile([P, TT, N_EXP], f32, name="eq1a", tag="eq1a", bufs=1)
            nc.vector.tensor_tensor(
                eq1a[:], logits_all[:], b3(m1a[:]), op=OP.is_equal)
            mska = gpool.tile([P, TT, N_EXP], f32, name="mska", tag="mska", bufs=1)
            nc.vector.scalar_tensor_tensor(
                mska[:], in0=eq1a[:], scalar=-1e30, in1=logits_all[:],
                op0=OP.mult, op1=OP.add)
            m2a = gpool.tile([P, TT], f32, name="m2a", tag="m2a", bufs=1)
            nc.vector.reduce_max(m2a[:], mska[:], axis=AX.X)
            eq2a = gpool.tile([P, TT, N_EXP], f32, name="eq2a", tag="eq2a", bufs=1)
            nc.vector.tensor_tensor(
                eq2a[:], mska[:], b3(m2a[:]), op=OP.is_equal)
            dlta = gpool.tile([P, TT], f32, name="dlta", tag="dlta", bufs=1)
            nc.vector.tensor_tensor(dlta[:], m2a[:], m1a[:], op=OP.subtract)
            p2a = gpool.tile([P, TT], f32, name="p2a", tag="p2a", bufs=1)
            nc.scalar.activation(p2a[:], dlta[:], ACT.Sigmoid)
            p1a = gpool.tile([P, TT], f32, name="p1a", tag="p1a", bufs=1)
            nc.vector.tensor_scalar(
                p1a[:], p2a[:], -1.0, 1.0, op0=OP.mult, op1=OP.add)
            g1a = gpool.tile([P, TT, N_EXP], f32, name="g1a", tag="g1a", bufs=1)
            nc.vector.tensor_mul(g1a[:], eq1a[:], b3(p1a[:]))
            nc.vector.tensor_mul(eq2a[:], eq2a[:], b3(p2a[:]))
            nc.vector.tensor_add(gate_sb[:], g1a[:], eq2a[:])

            # ---- experts, software-pipelined: mm1(e+1) is emitted between
            # the gating block and mm2(e) so the gate-chain latency hides
            # behind independent matmul work.
            ht_cur, w2_cur = ht_sb0, w2_sb0
            for e in range(N_EXP):
                if e + 1 < N_EXP:
                    w1_sb = wpool.tile([P, KC, C_HID], f16, name="w1_sb", tag="w1")
                    nc.sync.dma_start(
                        w1_sb[:], w1[e + 1].rearrange("(kc p) c -> p kc c", p=P)
                    )
                    w2_nxt = wpool.tile([P, CC, D_MODEL], f16, name="w2_sb", tag="w2")
                    nc.sync.dma_start(
                        w2_nxt[:], w2[e + 1].rearrange("(cc p) d -> p cc d", p=P)
                    )
                    ht_nxt = emit_mm1(w1_sb)
                else:
                    ht_nxt = w2_nxt = None
                emit_mm2(e, w2_cur, ht_cur)
                ht_cur, w2_cur = ht_nxt, w2_nxt

            for tt in range(TT):
                nc.sync.dma_start(
                    out[tt * P:(tt + 1) * P, :], out_sb[:, tt, :])

    _split_excess_waits(nc, mybir)
    return nc


def _get_nc():
    if "nc" not in _CACHE:
        _CACHE["nc"] = _build_nc()
    return _CACHE["nc"]


def kernel(**inputs) -> np.ndarray:
    global LAST_RESULT
    x = np.ascontiguousarray(np.asarray(inputs["x"], dtype=np.float32))
    Wg = np.ascontiguousarray(np.asarray(inputs["Wg"], dtype=np.float32))
    W1 = np.ascontiguousarray(np.asarray(inputs["W1"], dtype=np.float32))
    W2 = np.ascontiguousarray(np.asarray(inputs["W2"], dtype=np.float32))

    B, S, D = x.shape
    xf = x.reshape(B * S, D)
    w1h = np.ascontiguousarray(W1.astype(np.float16))
    w2h = np.ascontiguousarray(W2.astype(np.float16))
    wg16c = Wg.astype(np.float16)
    dwgc = (Wg - wg16c.astype(np.float32)).astype(np.float16)
    wgpc = np.ascontiguousarray(np.concatenate([wg16c, dwgc], axis=1))
    in_maps = []
    for i in range(N_CORES):
        shard = xf[i * TC:(i + 1) * TC]
        xt = np.ascontiguousarray(shard.T)
        xt16 = np.ascontiguousarray(xt.astype(np.float16))
        in_maps.append({
            "xT": xt16,
            "dxT": np.ascontiguousarray(
                (xt - xt16.astype(np.float32)).astype(np.float16)),
            "wgp": wgpc,
            "id8": np.eye(2 * N_EXP, dtype=np.float32),
            "w1": w1h,
            "w2": w2h,
        })

    from concourse.bass_utils import run_bass_kernel_spmd

    _install_ntff_hook_shim()
    nc = _get_nc()
    res = run_bass_kernel_spmd(
        nc, in_maps, core_ids=list(range(N_CORES)), trace=TRACE
    )
    LAST_RESULT = res
    out = np.concatenate([r["out"] for r in res.results], axis=0)
    return out.reshape(B, S, D)



# revision 2
# speedup vs baseline: 2.9510x; 2.9510x over previous
"""MoE layer (top-2 of 8 experts, d_model=1024, d_hidden=512) on 8 trn2 cores.

Expert-parallel with host-side dispatch/combine: the host computes gating in
float64 (exact routing), gathers each expert's assigned tokens into a padded
[D, CAP] buffer, and each core runs a single expert's 2-layer MLP in fp16
(fp32 PSUM accumulate). The gate probability is folded in on-device as a
per-partition scalar multiply during mm2, so each core outputs gate-weighted
token contributions; the host combines a token's two expert contributions
with two fancy-indexed gathers and an add (p1+p2=1, experts distinct, so the
math matches the reference exactly up to fp16 rounding in the MLP).

This computes only the top-2 selected expert-token pairs (1/4 of the dense
reference einsum), so per-core PE work is 2*CAP*D*C fp16 MACs ~ 61us.

Layout notes:
  - xgT arrives host-gathered+transposed per-expert as [D, CAP] so mm1
    contracts over the partition dimension with w1 in native layout.
  - mm1 produces hT [C, tokens] (w1 stationary); mm2 flips back to
    token-major y [tokens, D] (hT chunks stationary) so the gate is a
    per-partition [128,1] scalar and the output DMAs out token-major.
  - Output is fp16 (halves out-DMA); host upcasts and adds in fp32.
"""

import os
import sys

import numpy as np

for _p in ("/opt/trn_rl_repo", "/root/.axon_site/_ro/trn_rl_repo"):
    if _p not in sys.path and os.path.isdir(_p):
        sys.path.append(_p)

P = 128
D_MODEL = 1024
C_HID = 512
N_EXP = 8
N_CORES = 8
T_FULL = 4 * 2048

KC = D_MODEL // P  # 8 contraction chunks over D
CC = C_HID // P    # 4 contraction chunks over C
NT = 512           # moving-dim chunk (tokens) for mm1
DH = 512           # moving-dim chunk (d_model) for mm2

_CACHE = {}

# set by test harness to capture profiling info
TRACE = False
LAST_RESULT = None


def _install_ntff_hook_shim():
    """Register the axon NTFF profile hook if the image's antenv lacks it.

    bass_utils resolves the hook via `antenv.axon_hooks`; when that module is
    absent, tracing silently degrades. The hook implementation itself ships
    with the axon boot package, so wire it up through sys.modules.
    """
    try:
        from antenv.axon_hooks import get_axon_ntff_profile_hook  # noqa: F401
        return  # real module present
    except ImportError:
        pass
    try:
        import types

        if "/root/.axon_site" not in sys.path and os.path.isdir("/root/.axon_site"):
            sys.path.append("/root/.axon_site")
        from trn_agent_boot.trn_boot import _ntff_profile_via_ctypes

        so_path = "/opt/axon/libaxon_pjrt.so"
        if not os.path.exists(so_path):
            return
        hook = _ntff_profile_via_ctypes(so_path)
        mod = types.ModuleType("antenv.axon_hooks")
        mod.get_axon_ntff_profile_hook = lambda: hook
        mod.set_axon_ntff_profile_hook = lambda h: None
        import antenv

        antenv.axon_hooks = mod
        sys.modules["antenv.axon_hooks"] = mod
    except Exception:
        pass


def _split_excess_waits(nc, mybir, maxw=1):
    """This walrus build accepts at most one semaphore wait per instruction.

    Tile emits instructions (notably the kernel-tail drain) with several
    waits; split the extras into preceding single-wait NoOps on the same
    engine — program order makes the chain equivalent.
    """
    for f in nc.m.functions:
        for bb in f.blocks:
            out = []
            changed = False
            for ins in bb.instructions:
                si = ins.sync_info
                waits = list(si.on_wait) if (si is not None and si.on_wait) else []
                if len(waits) > maxw:
                    extra, keep = waits[:-maxw], waits[-maxw:]
                    for ci in range(0, len(extra), maxw):
                        out.append(mybir.InstNoOp(
                            name=f"{ins.name}_ws{ci}",
                            sync_info=mybir.SyncInfo(
                                on_wait=list(extra[ci:ci + maxw]), on_update=[]
                            ),
                            engine=ins.engine,
                            bass_nofuse=True,
                        ))
                    si.on_wait = keep
                    changed = True
                out.append(ins)
            if changed:
                bb.instructions = out
    return nc


def _build_nc(cap):
    import concourse.bass as bass
    import concourse.mybir as mybir
    import concourse.tile as tile
    from contextlib import ExitStack

    dt = mybir.dt
    f32 = dt.float32
    f16 = dt.float16
    OP = mybir.AluOpType
    ACT = mybir.ActivationFunctionType

    TT = cap // P  # token blocks of 128
    # mm1 token chunks: full NT-sized plus one tail
    chunks = []
    off = 0
    while off < cap:
        n = min(NT, cap - off)
        chunks.append((off, n))
        off += n

    nc = bass.Bass("TRN2", debug=False)

    xgT = nc.dram_tensor("xgT", [D_MODEL, cap], f16, kind="ExternalInput")
    w1 = nc.dram_tensor("w1", [D_MODEL, C_HID], f16, kind="ExternalInput")
    w2 = nc.dram_tensor("w2", [C_HID, D_MODEL], f16, kind="ExternalInput")
    g2d = nc.dram_tensor("g2d", [P, TT], f32, kind="ExternalInput")
    out = nc.dram_tensor("out", [cap, D_MODEL], f16, kind="ExternalOutput")

    with tile.TileContext(nc) as tc:
        with ExitStack() as ctx:
            cpool = ctx.enter_context(tc.tile_pool(name="cpool", bufs=1))
            opool = ctx.enter_context(tc.tile_pool(name="opool", bufs=3))
            psum_mm = ctx.enter_context(
                tc.tile_pool(name="psum_mm", bufs=4, space="PSUM"))

            xg_sb = cpool.tile([P, KC, cap], f16, name="xg_sb")
            w1_sb = cpool.tile([P, KC, C_HID], f16, name="w1_sb")
            w2_sb = cpool.tile([P, CC, D_MODEL], f16, name="w2_sb")
            ht_sb = cpool.tile([P, CC, cap], f16, name="ht_sb")
            g_sb = cpool.tile([P, TT], f32, name="g_sb")

            # DMA order tuned for earliest PE start: w1 and the first token
            # chunk feed mm1's first psum; the rest streams in behind it.
            nc.sync.dma_start(w1_sb[:], w1[:].rearrange("(kc p) c -> p kc c", p=P))
            o0, n0 = chunks[0]
            nc.sync.dma_start(
                xg_sb[:, :, o0:o0 + n0],
                xgT[:, o0:o0 + n0].rearrange("(kc p) t -> p kc t", p=P))
            for o, n in chunks[1:]:
                nc.sync.dma_start(
                    xg_sb[:, :, o:o + n],
                    xgT[:, o:o + n].rearrange("(kc p) t -> p kc t", p=P))
            nc.sync.dma_start(w2_sb[:], w2[:].rearrange("(cc p) d -> p cc d", p=P))
            nc.sync.dma_start(g_sb[:], g2d[:])

            # ---- mm1: hT[C, tokens] = relu(w1.T @ xgT), contract over D
            for o, n in chunks:
                for cm in range(CC):
                    ps_h = psum_mm.tile([P, NT], f32, name="ps_h", tag="ps")
                    for kc in range(KC):
                        nc.tensor.matmul(
                            ps_h[:, 0:n],
                            lhsT=w1_sb[:, kc, cm * P:(cm + 1) * P],
                            rhs=xg_sb[:, kc, o:o + n],
                            start=(kc == 0),
                            stop=(kc == KC - 1),
                        )
                    nc.scalar.activation(
                        ht_sb[:, cm, o:o + n], ps_h[:, 0:n], ACT.Relu)

            # ---- mm2: y[tokens, D] = g * (hT.T @ w2), contract over C;
            # token-major so the gate is a per-partition scalar and the
            # output DMAs out in native layout.
            for tt in range(TT):
                o_t = opool.tile([P, D_MODEL], f16, name="o_t", tag="o_t")
                for dh in range(D_MODEL // DH):
                    ps_y = psum_mm.tile([P, DH], f32, name="ps_y", tag="ps")
                    for cc in range(CC):
                        nc.tensor.matmul(
                            ps_y[:],
                            lhsT=ht_sb[:, cc, tt * P:(tt + 1) * P],
                            rhs=w2_sb[:, cc, dh * DH:(dh + 1) * DH],
                            start=(cc == 0),
                            stop=(cc == CC - 1),
                        )
                    nc.vector.tensor_single_scalar(
                        o_t[:, dh * DH:(dh + 1) * DH], ps_y[:],
                        g_sb[:, tt:tt + 1], op=OP.mult)
                nc.sync.dma_start(out[tt * P:(tt + 1) * P, :], o_t[:])

    import concourse.mybir as mybir2
    _split_excess_waits(nc, mybir2)
    return nc


def _get_nc(cap):
    key = ("nc", cap)
    if key not in _CACHE:
        _CACHE[key] = _build_nc(cap)
    return _CACHE[key]


def kernel(**inputs) -> np.ndarray:
    global LAST_RESULT
    x = np.asarray(inputs["x"], dtype=np.float32)
    Wg = np.asarray(inputs["Wg"], dtype=np.float32)
    W1 = np.asarray(inputs["W1"], dtype=np.float32)
    W2 = np.asarray(inputs["W2"], dtype=np.float32)

    B, S, D = x.shape
    T = B * S
    xf = x.reshape(T, D)

    # ---- routing on host in float64 (logit gaps >> fp32 matmul noise, so
    # this reproduces the reference's fp32 top-2 decisions exactly)
    logits = xf.astype(np.float64) @ Wg.astype(np.float64)
    rows = np.arange(T)
    i1 = logits.argmax(1)
    l1 = logits[rows, i1]
    masked = logits.copy()
    masked[rows, i1] = -np.inf
    i2 = masked.argmax(1)
    l2 = masked[rows, i2]
    p2 = 1.0 / (1.0 + np.exp(l1 - l2))
    p1 = 1.0 - p2

    # ---- dispatch: per-expert token lists + slot of each token in its
    # expert's buffer (for the combine gather)
    slot1 = np.empty(T, np.int64)
    slot2 = np.empty(T, np.int64)
    idx_e = []
    gates_e = []
    for e in range(N_EXP):
        a = np.nonzero(i1 == e)[0]
        b = np.nonzero(i2 == e)[0]
        slot1[a] = np.arange(len(a))
        slot2[b] = len(a) + np.arange(len(b))
        idx_e.append(np.concatenate([a, b]))
        gates_e.append(np.concatenate([p1[a], p2[b]]).astype(np.float32))
    max_n = max(len(ix) for ix in idx_e)
    cap = -(-max_n // P) * P
    TT = cap // P

    in_maps = []
    for e in range(N_EXP):
        n_e = len(idx_e[e])
        xg = np.zeros((D_MODEL, cap), np.float16)
        xg[:, :n_e] = xf[idx_e[e]].T.astype(np.float16)
        g2d = np.zeros((TT, P), np.float32)
        g2d.reshape(-1)[:n_e] = gates_e[e]
        in_maps.append({
            "xgT": np.ascontiguousarray(xg),
            "w1": np.ascontiguousarray(W1[e].astype(np.float16)),
            "w2": np.ascontiguousarray(W2[e].astype(np.float16)),
            "g2d": np.ascontiguousarray(g2d.T),
        })

    from concourse.bass_utils import run_bass_kernel_spmd

    _install_ntff_hook_shim()
    nc = _get_nc(cap)
    res = run_bass_kernel_spmd(
        nc, in_maps, core_ids=list(range(N_CORES)), trace=TRACE
    )
    LAST_RESULT = res

    # ---- combine: token t = p1*y[e1] + p2*y[e2]; gates already applied
    # on-device, so this is two gathers and an add.
    y_all = np.stack([r["out"] for r in res.results])  # [E, cap, D] f16
    yflat = y_all.reshape(N_EXP * cap, D_MODEL).astype(np.float32)
    out = yflat[i1 * cap + slot1] + yflat[i2 * cap + slot2]
    return out.reshape(B, S, D)
